# revision 12
# baseline (speedup 1.0000x reference)
"""Cross-attention TRN2 Bass kernel (nn_CrossAttention).

Full-input contract: kernel(**inputs) takes the unsharded numpy inputs and
returns the full output. Internally shards across 8 NeuronCores:
  core c -> batch b = c // 4, heads h0 = (c % 4) * 4 .. h0+3  (B=2, H=16)

v2: the baseline was ACT-throughput-walled (~147us of exp on the scalar
engine).  This version splits softmax-exp across TWO engines per attention
unit's 16 d-tiles:
  - 12 tiles: exact exp on ACT (PSUM->SBUF bf16, ~1.03us/tile)
  - 4 tiles (DVE_D): Schraudolph exp in bf16 on DVE via one tensor_scalar:
      i16 = round(x * 2^7/ln2 + (127*2^7 - 5.59)); bitcast i16 -> bf16
    max rel err ~3.3% per element, ~1.3e-2 end-to-end (gate 2e-2).
  All ae tiles are bf16 so A@V uses the fast all-bf16 path (163ns/MM,
  vs 211 f32r).  Denominator rides as a 65th ones-column of v (free:
  A@V cost is N-bound).
Inputs ship as fp16 (10 mantissa bits ~ f32r's 11; halves the DMA bytes --
the 8-core runs contend for shared HBM bandwidth, so DMA volume matters
more than the 1-core 72us suggests).  q/k/v projections consume fp16
directly; qT/kTp/v stay f32r/bf16 in SBUF as before.
Output DMA triggers ride the otherwise-idle Pool queue -- on the ACT queue
they head-of-line-blocked the exp stream (each trigger waits on its DVE
copy, stalling every later exp; measured +80us).
Everything else follows the baseline: one serial input-DMA queue in priority
order, f32r q/k projections, row-paired K=64 QK^T, projection work
interleaved into the attention units to track DMA arrival.
Host: normalize by the denominator row, add v bias, reassemble the
reference's raw (B, H*Dy*pd) reshape.
"""

import numpy as np

import concourse.bacc as bacc
import concourse.tile as tile
from concourse import mybir
from concourse.bass_utils import run_bass_kernel_spmd

DIM = 1024
H = 16
B = 2
SEQ = 2048  # both SEQ_X and SEQ_Y
PD = 64  # head dim
HPC = 4  # heads per core
PCOLS = HPC * PD  # 256 projection columns per core
N_CORES = 8

F32 = mybir.dt.float32
F32R = mybir.dt.float32r
BF16 = mybir.dt.bfloat16
I16 = mybir.dt.int16
F16 = mybir.dt.float16

# d-tiles whose exp runs on DVE (Schraudolph); rest on ACT (exact)
DVE_D = (3, 6, 9, 12, 15)
OUT_Q = "gpsimd"  # engine queue for output DMA triggers
AEB_BUFS = 6
AEF_BUFS = 4
EXPA = float(2.0 ** 7 / np.log(2.0))
EXPB = float(127.0 * 2 ** 7 - 366393.0 / 65536.0)

_NC_CACHE = None


def _round_f32r(a: np.ndarray) -> np.ndarray:
    """Round fp32 -> float32r bit pattern (RNE, drop low 12 mantissa bits)."""
    b = np.ascontiguousarray(a, dtype=np.float32).view(np.uint32).astype(np.uint64)
    half = np.uint64(1 << 11)
    lsb_mask = np.uint64((1 << 12) - 1)
    rounded = (b + half - np.uint64(1) + ((b >> np.uint64(12)) & np.uint64(1))) & ~lsb_mask
    return rounded.astype(np.uint32).view(np.float32).reshape(a.shape)


def _build_nc(repeat=1, loop_n=0):
    nc = bacc.Bacc(trn_type="TRN2", name="cross_attention")

    yt = nc.dram_tensor("yt", [DIM, SEQ], F16, kind="ExternalInput")
    xt = nc.dram_tensor("xt", [DIM, SEQ], F16, kind="ExternalInput")
    wqt = nc.dram_tensor("wqt", [DIM, PCOLS], F16, kind="ExternalInput")
    wkt = nc.dram_tensor("wkt", [DIM, PCOLS], F16, kind="ExternalInput")
    wvt = nc.dram_tensor("wvt", [DIM, PCOLS], F16, kind="ExternalInput")
    bq = nc.dram_tensor("bq", [PCOLS, 1], F32, kind="ExternalInput")
    o = nc.dram_tensor("o", [HPC, PD + 1, SEQ], F32, kind="ExternalOutput")

    NC = DIM // 128  # 8 c-tiles
    ND = SEQ // 128  # 16 d-tiles
    YC = 512  # attention y-chunk
    NY = SEQ // YC  # 4 y-chunks
    HY = SEQ // 2  # 1024 (projection y-half)

    with tile.TileContext(nc) as tc:
        with (
            tc.tile_pool(name="persist", bufs=1) as pp,
            tc.tile_pool(name="wts", bufs=2) as wtp,
            tc.tile_pool(name="xtp", bufs=2) as xtp,
            tc.tile_pool(name="ytp", bufs=4) as ytp,
            tc.tile_pool(name="aeb", bufs=AEB_BUFS) as aebp,
            tc.tile_pool(name="aef", bufs=AEF_BUFS) as aefp,
            tc.tile_pool(name="outp", bufs=4) as outp,
            tc.tile_pool(name="ps_att", bufs=3, space="PSUM") as ps_att,
            tc.tile_pool(name="ps_o", bufs=2, space="PSUM") as ps_o,
        ):
            if loop_n:
                loop_cm = tc.For_i(0, loop_n, 1)
            else:
                loop_cm = None
            with (loop_cm if loop_cm is not None else __import__("contextlib").nullcontext()):
              for rep in range(repeat):
                # ---- resident loads ----
                # DMA behaves as ONE serial ~250GB/s resource: put every input
                # on the sync queue in exact priority order.  Outputs go on the
                # scalar queue.
                yt_src = yt.ap().rearrange("(c p) s -> p c s", p=128)
                xt_src = xt.ap().rearrange("(c p) s -> p c s", p=128)
                xt_e = [None] * 4

                def emit_xt(e):
                    t = xtp.tile([128, NC, 512], F16, tag=f"xte{e}", name=f"xte{e}_r{rep}")
                    nc.sync.dma_start(out=t, in_=xt_src[:, :, e * 512:(e + 1) * 512])
                    xt_e[e] = t

                wq_big = wtp.tile([128, NC, PCOLS], F16, tag="wq", name=f"wq_r{rep}")
                nc.sync.dma_start(
                    out=wq_big, in_=wqt.ap().rearrange("(c p) n -> p c n", p=128))
                # yt as four [128, 8, 512] y-chunk transfers; yt_cur2[yh][j]
                yt_cur2 = [[None, None], [None, None]]

                def emit_yt(yh, jj):
                    t = ytp.tile([128, NC, YC], F16, tag="yt", name=f"yt{yh}_{jj}_r{rep}")
                    lo = yh * HY + jj * YC
                    nc.sync.dma_start(out=t, in_=yt_src[:, :, lo:lo + YC])
                    yt_cur2[yh][jj] = t

                emit_yt(0, 0)
                wk_big = wtp.tile([128, NC, PCOLS], F16, tag="wk", name=f"wk_r{rep}")
                nc.sync.dma_start(
                    out=wk_big, in_=wkt.ap().rearrange("(c p) n -> p c n", p=128))
                emit_xt(0)
                emit_xt(1)
                bq_sb = []
                for m in range(2):
                    t = pp.tile([128, 1], F32, tag=f"bq{m}", name=f"bq{m}_r{rep}")
                    nc.sync.dma_start(out=t, in_=bq.ap()[m * 128:(m + 1) * 128, :])
                    bq_sb.append(t)
                wv_big = wtp.tile([128, NC, PCOLS], F16, tag="wv", name=f"wv_r{rep}")
                nc.sync.dma_start(
                    out=wv_big, in_=wvt.ap().rearrange("(c p) n -> p c n", p=128))
                emit_xt(2)
                emit_xt(3)
                emit_yt(0, 1)
                emit_yt(1, 0)
                emit_yt(1, 1)

                def xt_slice(c, lo, hi):
                    e = lo // 512
                    assert hi <= (e + 1) * 512
                    return xt_e[e][:, c, lo - e * 512:hi - e * 512]

                qT_sb = [pp.tile([128, SEQ], F32R, tag=f"qT{m}", name=f"qT{m}_r{rep}") for m in range(2)]
                # kT per head pair: rows 0-63 = even head, 64-127 = odd head
                kTp = [pp.tile([128, SEQ], F32R, tag=f"kp{m}", name=f"kp{m}_r{rep}") for m in range(2)]
                # v per d-tile, bf16, 65th column = 1.0 (denominator row)
                v_bf = [pp.tile([128, HPC, PD + 1], BF16, tag=f"vb{d}", name=f"vb{d}_r{rep}") for d in range(ND)]
                ones_sb = pp.tile([128, HPC], F32, tag="ones", name=f"ones_r{rep}")
                nc.vector.memset(ones_sb, 1.0)
                for d in range(ND):
                    nc.vector.tensor_copy(v_bf[d][:, :, PD:PD + 1], ones_sb)

                # Projection psum tiles share the "pa" slots of ps_att.
                def proj_q(m, yh, jj, cs=None, ps=None):
                    """q projection for one y-512 chunk jj of half yh."""
                    if ps is None:
                        ps = ps_att.tile([128, YC], F32, tag="pa", name=f"pq{m}_{yh}_{jj}_r{rep}")
                    for c in (cs if cs is not None else range(NC)):
                        nc.tensor.matmul(
                            ps,
                            wq_big[:, c, m * 128:(m + 1) * 128],
                            yt_cur2[yh][jj][:, c, :],
                            start=(c == 0),
                            stop=(c == NC - 1),
                        )
                    if cs is not None and list(cs)[-1] != NC - 1:
                        return ps
                    lo = yh * HY + jj * YC
                    nc.vector.tensor_scalar_add(qT_sb[m][:, lo:lo + YC], ps, bq_sb[m])
                    return ps

                def proj_split_q(m, yh, jj, d0, interleave, nq=2):
                    state = {"ps": None}
                    w = NC // nq

                    def chunk(i):
                        def f():
                            state["ps"] = proj_q(
                                m, yh, jj, cs=range(i * w, (i + 1) * w),
                                ps=state["ps"])
                        return f

                    for i in range(nq):
                        interleave.setdefault(d0 + i, []).append(chunk(i))

                def proj_pass_e(m, e, cs=None, ps=None):
                    """[128, 512] k projection chunk-pass: head pair m, x e-chunk."""
                    if ps is None:
                        ps = ps_att.tile([128, 512], F32, tag="pa", name=f"pk{m}_{e}_r{rep}")
                    for c in (cs if cs is not None else range(NC)):
                        nc.tensor.matmul(
                            ps,
                            wk_big[:, c, m * 128:(m + 1) * 128],
                            xt_e[e][:, c, :],
                            start=(c == 0),
                            stop=(c == NC - 1),
                        )
                    if cs is not None and list(cs)[-1] != NC - 1:
                        return ps
                    nc.vector.tensor_copy(kTp[m][:, e * 512:(e + 1) * 512], ps)
                    return ps

                def proj_pass(kind, m, yh):
                    proj_pass_e(m, 2 * yh)
                    proj_pass_e(m, 2 * yh + 1)

                def proj_split(kind, m, yh, d0, interleave, nq=4):
                    """Emit a k half-pass as nq chunk-passes at steps d0.."""
                    state = {"ps0": None, "ps1": None}

                    def chunk(e_off, i):
                        def f():
                            key = f"ps{e_off}"
                            state[key] = proj_pass_e(
                                m, 2 * yh + e_off, cs=range(i * 4, (i + 1) * 4),
                                ps=state[key])
                        return f

                    assert nq == 4
                    for i in range(2):
                        interleave.setdefault(d0 + i, []).append(chunk(0, i))
                    for i in range(2):
                        interleave.setdefault(d0 + 2 + i, []).append(chunk(1, i))

                def proj_v_single(d):
                    pvt = ps_att.tile([128, PCOLS], F32, tag="pa", name=f"pv{d}_r{rep}")
                    for c in range(NC):
                        nc.tensor.matmul(
                            pvt,
                            xt_slice(c, d * 128, (d + 1) * 128),
                            wv_big[:, c, :],
                            start=(c == 0),
                            stop=(c == NC - 1),
                        )
                    nc.vector.tensor_copy(
                        v_bf[d][:, :, 0:PD],
                        pvt.rearrange("p (h e) -> p h e", h=HPC),
                    )

                def attention_unit(m, yc, interleave=None, av_lag=4):
                    """One (head pair, y-chunk of 512): row-tiled QK^T -> exp
                    (ACT or DVE by d-tile) -> A@V for both heads."""
                    po_a = ps_o.tile([PD + 1, YC], F32, tag="po", name=f"poA{m}_{yc}_r{rep}")
                    po_b = ps_o.tile([PD + 1, YC], F32, tag="po", name=f"poB{m}_{yc}_r{rep}")
                    y0 = yc * YC
                    ae_q = {}

                    def emit_av(d):
                        v = v_bf[d]
                        nc.tensor.matmul(
                            po_a,
                            v[:, 2 * m, :],
                            ae_q[d][:, 0:YC],
                            start=(d == 0), stop=(d == ND - 1),
                        )
                        nc.tensor.matmul(
                            po_b,
                            v[:, 2 * m + 1, :],
                            ae_q[d][:, YC:2 * YC],
                            start=(d == 0), stop=(d == ND - 1),
                        )
                        del ae_q[d]

                    for d in range(ND):
                        if interleave and d in interleave:
                            fns = interleave[d]
                            for fn in (fns if isinstance(fns, (list, tuple)) else [fns]):
                                fn()
                        pa = ps_att.tile([128, 2 * YC], F32, tag="pa", name=f"pa{m}_{d}_{yc}_r{rep}")
                        nc.tensor.matmul(
                            pa[:, 0:YC],
                            kTp[m][0:64, d * 128:(d + 1) * 128],
                            qT_sb[m][0:64, y0:y0 + YC],
                            start=True, stop=True,
                        )
                        nc.tensor.matmul(
                            pa[:, YC:2 * YC],
                            kTp[m][64:128, d * 128:(d + 1) * 128],
                            qT_sb[m][64:128, y0:y0 + YC],
                            start=True, stop=True,
                        )
                        if d in DVE_D:
                            ae = aefp.tile([128, 2 * YC], BF16, tag="aef", name=f"aef{m}_{d}_{yc}_r{rep}")
                            nc.vector.tensor_scalar(
                                ae.bitcast(I16), pa, EXPA, EXPB,
                                mybir.AluOpType.mult, mybir.AluOpType.add)
                        else:
                            ae = aebp.tile([128, 2 * YC], BF16, tag="aeb", name=f"aeb{m}_{d}_{yc}_r{rep}")
                            nc.scalar.activation(
                                out=ae,
                                in_=pa,
                                func=mybir.ActivationFunctionType.Exp,
                                scale=1.0,
                            )
                        ae_q[d] = ae
                        if d >= av_lag:
                            emit_av(d - av_lag)
                    for d in range(ND - av_lag, ND):
                        emit_av(d)
                    for j, po in ((0, po_a), (1, po_b)):
                        h = 2 * m + j
                        osb = outp.tile([PD + 1, YC], F32, tag="osb", name=f"osb{h}_{yc}_r{rep}")
                        nc.vector.tensor_copy(osb, po)
                        getattr(nc, OUT_Q).dma_start(
                            out=o.ap()[h, :, y0:y0 + YC], in_=osb)

                # ---- emission order drives scheduling priority ----
                # Unit order (0,0),(1,0),(0,1),(1,1),... so work needing the
                # second x half / second y chunks lands after its DMA.
                proj_q(0, 0, 0)
                proj_q(1, 0, 0)
                proj_pass("k", 0, 0)
                il0 = {}
                for d in range(8):
                    il0.setdefault(d, []).append(lambda d=d: proj_v_single(d))
                proj_split("k", 0, 1, 4, il0)      # x half 1 (xt1)
                proj_split("k", 1, 0, 8, il0)      # x half 0
                for d in range(8, 13):
                    il0.setdefault(d + 3, []).append(lambda d=d: proj_v_single(d))
                for d in range(13, ND):
                    il0.setdefault(15, []).append(lambda d=d: proj_v_single(d))
                attention_unit(0, 0, interleave=il0, av_lag=3)
                il1 = {}
                proj_split("k", 1, 1, 0, il1)      # x half 1
                proj_split_q(0, 0, 1, 4, il1)      # y chunk 1 (yt0 j1)
                proj_split_q(1, 0, 1, 6, il1)
                attention_unit(1, 0, interleave=il1)
                il2 = {}
                proj_split_q(0, 1, 0, 2, il2)      # y half 1 (yt1 j0)
                proj_split_q(1, 1, 0, 4, il2)
                attention_unit(0, 1, interleave=il2)
                il3 = {}
                proj_split_q(0, 1, 1, 2, il3)
                proj_split_q(1, 1, 1, 4, il3)
                attention_unit(1, 1, interleave=il3)
                attention_unit(0, 2)
                attention_unit(1, 2)
                attention_unit(0, 3)
                attention_unit(1, 3)

    nc.compile()
    return nc


def _get_nc():
    global _NC_CACHE
    if _NC_CACHE is None:
        _NC_CACHE = _build_nc()
    return _NC_CACHE


_NC_LOOP_CACHE = {}


def _get_nc_loop(loop_n):
    if loop_n not in _NC_LOOP_CACHE:
        _NC_LOOP_CACHE[loop_n] = _build_nc(1, loop_n=loop_n)
    return _NC_LOOP_CACHE[loop_n]


def make_in_map(x, y, Wq, bq_np, Wkv, core):
    b = core // 4
    h0 = (core % 4) * HPC
    cs = slice(h0 * PD, h0 * PD + PCOLS)
    vs = slice(DIM + h0 * PD, DIM + h0 * PD + PCOLS)
    return {
        "yt": np.ascontiguousarray(y[b].T).astype(np.float16),
        "xt": np.ascontiguousarray(x[b].T).astype(np.float16),
        "wqt": np.ascontiguousarray(Wq[cs, :].T).astype(np.float16),
        "wkt": np.ascontiguousarray(Wkv[cs, :].T).astype(np.float16),
        "wvt": np.ascontiguousarray(Wkv[vs, :].T).astype(np.float16),
        "bq": np.ascontiguousarray(bq_np[cs].reshape(PCOLS, 1)).astype(np.float32),
    }


def kernel(x, y, Wq, bq, Wkv, bkv, _collect_results=None):
    x = np.asarray(x, dtype=np.float32)
    y = np.asarray(y, dtype=np.float32)
    Wq = np.asarray(Wq, dtype=np.float32)
    bq = np.asarray(bq, dtype=np.float32)
    Wkv = np.asarray(Wkv, dtype=np.float32)
    bkv = np.asarray(bkv, dtype=np.float32)

    nc = _get_nc()

    in_maps = [make_in_map(x, y, Wq, bq, Wkv, core) for core in range(N_CORES)]

    res = run_bass_kernel_spmd(nc, in_maps, list(range(N_CORES)))
    if _collect_results is not None:
        _collect_results.append(res)

    O = np.empty((B, H, SEQ, PD), np.float32)
    for core in range(N_CORES):
        b = core // 4
        h0 = (core % 4) * HPC
        oc = res.results[core]["o"]  # [HPC, PD+1, SEQ]
        num = oc[:, :PD, :].astype(np.float64)
        den = oc[:, PD, :].astype(np.float64)
        for i in range(HPC):
            h = h0 + i
            bv = bkv[DIM + h * PD:DIM + (h + 1) * PD]
            O[b, h] = (num[i] / den[i][None, :]).T + bv[None, :]
    return O.reshape(B, SEQ, DIM)


# revision 13
# speedup vs baseline: 1.0243x; 1.0243x over previous
"""Cross-attention TRN2 Bass kernel (nn_CrossAttention).

Full-input contract: kernel(**inputs) takes the unsharded numpy inputs and
returns the full output. Internally shards across 8 NeuronCores:
  core c -> batch b = c // 4, heads h0 = (c % 4) * 4 .. h0+3  (B=2, H=16)

v2: the baseline was ACT-throughput-walled (~147us of exp on the scalar
engine).  This version splits softmax-exp across TWO engines per attention
unit's 16 d-tiles:
  - 12 tiles: exact exp on ACT (PSUM->SBUF bf16, ~1.03us/tile)
  - 4 tiles (DVE_D): Schraudolph exp in bf16 on DVE via one tensor_scalar:
      i16 = round(x * 2^7/ln2 + (127*2^7 - 5.59)); bitcast i16 -> bf16
    max rel err ~3.3% per element, ~1.3e-2 end-to-end (gate 2e-2).
  All ae tiles are bf16 so A@V uses the fast all-bf16 path (163ns/MM,
  vs 211 f32r).  Denominator rides as a 65th ones-column of v (free:
  A@V cost is N-bound).
Inputs ship as fp16 (10 mantissa bits ~ f32r's 11; halves the DMA bytes --
the 8-core runs contend for shared HBM bandwidth, so DMA volume matters
more than the 1-core 72us suggests).  q/k/v projections consume fp16
directly; qT/kTp/v stay f32r/bf16 in SBUF as before.
Output DMA triggers ride the otherwise-idle Pool queue -- on the ACT queue
they head-of-line-blocked the exp stream (each trigger waits on its DVE
copy, stalling every later exp; measured +80us).
Everything else follows the baseline: one serial input-DMA queue in priority
order, f32r q/k projections, row-paired K=64 QK^T, projection work
interleaved into the attention units to track DMA arrival.
Host: normalize by the denominator row, add v bias, reassemble the
reference's raw (B, H*Dy*pd) reshape.
"""

import numpy as np

import concourse.bacc as bacc
import concourse.tile as tile
from concourse import mybir
from concourse.bass_utils import run_bass_kernel_spmd

DIM = 1024
H = 16
B = 2
SEQ = 2048  # both SEQ_X and SEQ_Y
PD = 64  # head dim
HPC = 4  # heads per core
PCOLS = HPC * PD  # 256 projection columns per core
N_CORES = 8

F32 = mybir.dt.float32
F32R = mybir.dt.float32r
BF16 = mybir.dt.bfloat16
I16 = mybir.dt.int16
F16 = mybir.dt.float16

# d-tiles whose exp runs on DVE (Schraudolph); rest on ACT (exact)
DVE_D = (3, 6, 9, 12, 15)
OUT_Q = "gpsimd"  # engine queue for output DMA triggers
AEB_BUFS = 6
AEF_BUFS = 4
EXPA = float(2.0 ** 7 / np.log(2.0))
EXPB = float(127.0 * 2 ** 7 - 366393.0 / 65536.0)

_NC_CACHE = None


def _round_f32r(a: np.ndarray) -> np.ndarray:
    """Round fp32 -> float32r bit pattern (RNE, drop low 12 mantissa bits)."""
    b = np.ascontiguousarray(a, dtype=np.float32).view(np.uint32).astype(np.uint64)
    half = np.uint64(1 << 11)
    lsb_mask = np.uint64((1 << 12) - 1)
    rounded = (b + half - np.uint64(1) + ((b >> np.uint64(12)) & np.uint64(1))) & ~lsb_mask
    return rounded.astype(np.uint32).view(np.float32).reshape(a.shape)


def _build_nc(repeat=1, loop_n=0):
    nc = bacc.Bacc(trn_type="TRN2", name="cross_attention")

    yt = nc.dram_tensor("yt", [DIM, SEQ], F16, kind="ExternalInput")
    xt = nc.dram_tensor("xt", [DIM, SEQ], F16, kind="ExternalInput")
    wqt = nc.dram_tensor("wqt", [DIM, PCOLS], F16, kind="ExternalInput")
    wkt = nc.dram_tensor("wkt", [DIM, PCOLS], F16, kind="ExternalInput")
    wvt = nc.dram_tensor("wvt", [DIM, PCOLS], F16, kind="ExternalInput")
    bq = nc.dram_tensor("bq", [PCOLS, 1], F32, kind="ExternalInput")
    o = nc.dram_tensor("o", [HPC, PD + 1, SEQ], F32, kind="ExternalOutput")

    NC = DIM // 128  # 8 c-tiles
    ND = SEQ // 128  # 16 d-tiles
    YC = 512  # attention y-chunk
    NY = SEQ // YC  # 4 y-chunks
    HY = SEQ // 2  # 1024 (projection y-half)

    with tile.TileContext(nc) as tc:
        with (
            tc.tile_pool(name="persist", bufs=1) as pp,
            tc.tile_pool(name="wts", bufs=2) as wtp,
            tc.tile_pool(name="xtp", bufs=2) as xtp,
            tc.tile_pool(name="ytp", bufs=4) as ytp,
            tc.tile_pool(name="aeb", bufs=AEB_BUFS) as aebp,
            tc.tile_pool(name="aef", bufs=AEF_BUFS) as aefp,
            tc.tile_pool(name="outp", bufs=4) as outp,
            tc.tile_pool(name="ps_att", bufs=3, space="PSUM") as ps_att,
            tc.tile_pool(name="ps_o", bufs=2, space="PSUM") as ps_o,
        ):
            if loop_n:
                loop_cm = tc.For_i(0, loop_n, 1)
            else:
                loop_cm = None
            with (loop_cm if loop_cm is not None else __import__("contextlib").nullcontext()):
              for rep in range(repeat):
                # ---- resident loads ----
                # DMA behaves as ONE serial ~250GB/s resource: put every input
                # on the sync queue in exact priority order.  Outputs go on the
                # scalar queue.
                yt_src = yt.ap().rearrange("(c p) s -> p c s", p=128)
                xt_src = xt.ap().rearrange("(c p) s -> p c s", p=128)
                xt_e = [None] * 4

                def emit_xt(e):
                    t = xtp.tile([128, NC, 512], F16, tag=f"xte{e}", name=f"xte{e}_r{rep}")
                    nc.sync.dma_start(out=t, in_=xt_src[:, :, e * 512:(e + 1) * 512])
                    xt_e[e] = t

                wq_big = wtp.tile([128, NC, PCOLS], F16, tag="wq", name=f"wq_r{rep}")
                nc.sync.dma_start(
                    out=wq_big, in_=wqt.ap().rearrange("(c p) n -> p c n", p=128))
                # yt as four [128, 8, 512] y-chunk transfers; yt_cur2[yh][j]
                yt_cur2 = [[None, None], [None, None]]

                def emit_yt(yh, jj):
                    t = ytp.tile([128, NC, YC], F16, tag="yt", name=f"yt{yh}_{jj}_r{rep}")
                    lo = yh * HY + jj * YC
                    nc.sync.dma_start(out=t, in_=yt_src[:, :, lo:lo + YC])
                    yt_cur2[yh][jj] = t

                emit_yt(0, 0)
                wk_big = wtp.tile([128, NC, PCOLS], F16, tag="wk", name=f"wk_r{rep}")
                nc.sync.dma_start(
                    out=wk_big, in_=wkt.ap().rearrange("(c p) n -> p c n", p=128))
                emit_xt(0)
                emit_xt(1)
                bq_sb = []
                for m in range(2):
                    t = pp.tile([128, 1], F32, tag=f"bq{m}", name=f"bq{m}_r{rep}")
                    nc.sync.dma_start(out=t, in_=bq.ap()[m * 128:(m + 1) * 128, :])
                    bq_sb.append(t)
                wv_big = wtp.tile([128, NC, PCOLS], F16, tag="wv", name=f"wv_r{rep}")
                nc.sync.dma_start(
                    out=wv_big, in_=wvt.ap().rearrange("(c p) n -> p c n", p=128))
                emit_xt(2)
                emit_xt(3)
                emit_yt(0, 1)
                emit_yt(1, 0)
                emit_yt(1, 1)

                def xt_slice(c, lo, hi):
                    e = lo // 512
                    assert hi <= (e + 1) * 512
                    return xt_e[e][:, c, lo - e * 512:hi - e * 512]

                qT_sb = [pp.tile([128, SEQ], F16, tag=f"qT{m}", name=f"qT{m}_r{rep}") for m in range(2)]
                # kT per head pair: rows 0-63 = even head, 64-127 = odd head
                kTp = [pp.tile([128, SEQ], F16, tag=f"kp{m}", name=f"kp{m}_r{rep}") for m in range(2)]
                # v per d-tile, bf16, 65th column = 1.0 (denominator row)
                v_bf = [pp.tile([128, HPC, PD + 1], BF16, tag=f"vb{d}", name=f"vb{d}_r{rep}") for d in range(ND)]
                ones_sb = pp.tile([128, HPC], F32, tag="ones", name=f"ones_r{rep}")
                nc.vector.memset(ones_sb, 1.0)
                for d in range(ND):
                    nc.vector.tensor_copy(v_bf[d][:, :, PD:PD + 1], ones_sb)

                # Projection psum tiles share the "pa" slots of ps_att.
                def proj_q(m, yh, jj, cs=None, ps=None):
                    """q projection for one y-512 chunk jj of half yh."""
                    if ps is None:
                        ps = ps_att.tile([128, YC], F32, tag="pa", name=f"pq{m}_{yh}_{jj}_r{rep}")
                    for c in (cs if cs is not None else range(NC)):
                        nc.tensor.matmul(
                            ps,
                            wq_big[:, c, m * 128:(m + 1) * 128],
                            yt_cur2[yh][jj][:, c, :],
                            start=(c == 0),
                            stop=(c == NC - 1),
                        )
                    if cs is not None and list(cs)[-1] != NC - 1:
                        return ps
                    lo = yh * HY + jj * YC
                    nc.vector.tensor_scalar_add(qT_sb[m][:, lo:lo + YC], ps, bq_sb[m])
                    return ps

                def proj_split_q(m, yh, jj, d0, interleave, nq=2):
                    state = {"ps": None}
                    w = NC // nq

                    def chunk(i):
                        def f():
                            state["ps"] = proj_q(
                                m, yh, jj, cs=range(i * w, (i + 1) * w),
                                ps=state["ps"])
                        return f

                    for i in range(nq):
                        interleave.setdefault(d0 + i, []).append(chunk(i))

                def proj_pass_e(m, e, cs=None, ps=None):
                    """[128, 512] k projection chunk-pass: head pair m, x e-chunk."""
                    if ps is None:
                        ps = ps_att.tile([128, 512], F32, tag="pa", name=f"pk{m}_{e}_r{rep}")
                    for c in (cs if cs is not None else range(NC)):
                        nc.tensor.matmul(
                            ps,
                            wk_big[:, c, m * 128:(m + 1) * 128],
                            xt_e[e][:, c, :],
                            start=(c == 0),
                            stop=(c == NC - 1),
                        )
                    if cs is not None and list(cs)[-1] != NC - 1:
                        return ps
                    nc.vector.tensor_copy(kTp[m][:, e * 512:(e + 1) * 512], ps)
                    return ps

                def proj_pass(kind, m, yh):
                    proj_pass_e(m, 2 * yh)
                    proj_pass_e(m, 2 * yh + 1)

                def proj_split(kind, m, yh, d0, interleave, nq=4):
                    """Emit a k half-pass as nq chunk-passes at steps d0.."""
                    state = {"ps0": None, "ps1": None}

                    def chunk(e_off, i):
                        def f():
                            key = f"ps{e_off}"
                            state[key] = proj_pass_e(
                                m, 2 * yh + e_off, cs=range(i * 4, (i + 1) * 4),
                                ps=state[key])
                        return f

                    assert nq == 4
                    for i in range(2):
                        interleave.setdefault(d0 + i, []).append(chunk(0, i))
                    for i in range(2):
                        interleave.setdefault(d0 + 2 + i, []).append(chunk(1, i))

                def proj_v_single(d):
                    pvt = ps_att.tile([128, PCOLS], F32, tag="pa", name=f"pv{d}_r{rep}")
                    for c in range(NC):
                        nc.tensor.matmul(
                            pvt,
                            xt_slice(c, d * 128, (d + 1) * 128),
                            wv_big[:, c, :],
                            start=(c == 0),
                            stop=(c == NC - 1),
                        )
                    nc.vector.tensor_copy(
                        v_bf[d][:, :, 0:PD],
                        pvt.rearrange("p (h e) -> p h e", h=HPC),
                    )

                def attention_unit(m, yc, interleave=None, av_lag=4):
                    """One (head pair, y-chunk of 512): row-tiled QK^T -> exp
                    (ACT or DVE by d-tile) -> A@V for both heads."""
                    po_a = ps_o.tile([PD + 1, YC], F32, tag="po", name=f"poA{m}_{yc}_r{rep}")
                    po_b = ps_o.tile([PD + 1, YC], F32, tag="po", name=f"poB{m}_{yc}_r{rep}")
                    y0 = yc * YC
                    ae_q = {}

                    def emit_av(d):
                        v = v_bf[d]
                        nc.tensor.matmul(
                            po_a,
                            v[:, 2 * m, :],
                            ae_q[d][:, 0:YC],
                            start=(d == 0), stop=(d == ND - 1),
                        )
                        nc.tensor.matmul(
                            po_b,
                            v[:, 2 * m + 1, :],
                            ae_q[d][:, YC:2 * YC],
                            start=(d == 0), stop=(d == ND - 1),
                        )
                        del ae_q[d]

                    for d in range(ND):
                        if interleave and d in interleave:
                            fns = interleave[d]
                            for fn in (fns if isinstance(fns, (list, tuple)) else [fns]):
                                fn()
                        pa = ps_att.tile([128, 2 * YC], F32, tag="pa", name=f"pa{m}_{d}_{yc}_r{rep}")
                        nc.tensor.matmul(
                            pa[:, 0:YC],
                            kTp[m][0:64, d * 128:(d + 1) * 128],
                            qT_sb[m][0:64, y0:y0 + YC],
                            start=True, stop=True,
                        )
                        nc.tensor.matmul(
                            pa[:, YC:2 * YC],
                            kTp[m][64:128, d * 128:(d + 1) * 128],
                            qT_sb[m][64:128, y0:y0 + YC],
                            start=True, stop=True,
                        )
                        if d in DVE_D:
                            ae = aefp.tile([128, 2 * YC], BF16, tag="aef", name=f"aef{m}_{d}_{yc}_r{rep}")
                            nc.vector.tensor_scalar(
                                ae.bitcast(I16), pa, EXPA, EXPB,
                                mybir.AluOpType.mult, mybir.AluOpType.add)
                        else:
                            ae = aebp.tile([128, 2 * YC], BF16, tag="aeb", name=f"aeb{m}_{d}_{yc}_r{rep}")
                            nc.scalar.activation(
                                out=ae,
                                in_=pa,
                                func=mybir.ActivationFunctionType.Exp,
                                scale=1.0,
                            )
                        ae_q[d] = ae
                        if d >= av_lag:
                            emit_av(d - av_lag)
                    for d in range(ND - av_lag, ND):
                        emit_av(d)
                    for j, po in ((0, po_a), (1, po_b)):
                        h = 2 * m + j
                        osb = outp.tile([PD + 1, YC], F32, tag="osb", name=f"osb{h}_{yc}_r{rep}")
                        nc.vector.tensor_copy(osb, po)
                        getattr(nc, OUT_Q).dma_start(
                            out=o.ap()[h, :, y0:y0 + YC], in_=osb)

                # ---- emission order drives scheduling priority ----
                # Unit order (0,0),(1,0),(0,1),(1,1),... so work needing the
                # second x half / second y chunks lands after its DMA.
                proj_q(0, 0, 0)
                proj_q(1, 0, 0)
                proj_pass("k", 0, 0)
                il0 = {}
                for d in range(8):
                    il0.setdefault(d, []).append(lambda d=d: proj_v_single(d))
                proj_split("k", 0, 1, 4, il0)      # x half 1 (xt1)
                proj_split("k", 1, 0, 8, il0)      # x half 0
                for d in range(8, 13):
                    il0.setdefault(d + 3, []).append(lambda d=d: proj_v_single(d))
                for d in range(13, ND):
                    il0.setdefault(15, []).append(lambda d=d: proj_v_single(d))
                attention_unit(0, 0, interleave=il0, av_lag=3)
                il1 = {}
                proj_split("k", 1, 1, 0, il1)      # x half 1
                proj_split_q(0, 0, 1, 4, il1)      # y chunk 1 (yt0 j1)
                proj_split_q(1, 0, 1, 6, il1)
                attention_unit(1, 0, interleave=il1)
                il2 = {}
                proj_split_q(0, 1, 0, 2, il2)      # y half 1 (yt1 j0)
                proj_split_q(1, 1, 0, 4, il2)
                attention_unit(0, 1, interleave=il2)
                il3 = {}
                proj_split_q(0, 1, 1, 2, il3)
                proj_split_q(1, 1, 1, 4, il3)
                attention_unit(1, 1, interleave=il3)
                attention_unit(0, 2)
                attention_unit(1, 2)
                attention_unit(0, 3)
                attention_unit(1, 3)

    nc.compile()
    return nc


def _get_nc():
    global _NC_CACHE
    if _NC_CACHE is None:
        _NC_CACHE = _build_nc()
    return _NC_CACHE


_NC_LOOP_CACHE = {}


def _get_nc_loop(loop_n):
    if loop_n not in _NC_LOOP_CACHE:
        _NC_LOOP_CACHE[loop_n] = _build_nc(1, loop_n=loop_n)
    return _NC_LOOP_CACHE[loop_n]


def make_in_map(x, y, Wq, bq_np, Wkv, core):
    b = core // 4
    h0 = (core % 4) * HPC
    cs = slice(h0 * PD, h0 * PD + PCOLS)
    vs = slice(DIM + h0 * PD, DIM + h0 * PD + PCOLS)
    return {
        "yt": np.ascontiguousarray(y[b].T).astype(np.float16),
        "xt": np.ascontiguousarray(x[b].T).astype(np.float16),
        "wqt": np.ascontiguousarray(Wq[cs, :].T).astype(np.float16),
        "wkt": np.ascontiguousarray(Wkv[cs, :].T).astype(np.float16),
        "wvt": np.ascontiguousarray(Wkv[vs, :].T).astype(np.float16),
        "bq": np.ascontiguousarray(bq_np[cs].reshape(PCOLS, 1)).astype(np.float32),
    }


def kernel(x, y, Wq, bq, Wkv, bkv, _collect_results=None):
    x = np.asarray(x, dtype=np.float32)
    y = np.asarray(y, dtype=np.float32)
    Wq = np.asarray(Wq, dtype=np.float32)
    bq = np.asarray(bq, dtype=np.float32)
    Wkv = np.asarray(Wkv, dtype=np.float32)
    bkv = np.asarray(bkv, dtype=np.float32)

    nc = _get_nc()

    in_maps = [make_in_map(x, y, Wq, bq, Wkv, core) for core in range(N_CORES)]

    res = run_bass_kernel_spmd(nc, in_maps, list(range(N_CORES)))
    if _collect_results is not None:
        _collect_results.append(res)

    O = np.empty((B, H, SEQ, PD), np.float32)
    for core in range(N_CORES):
        b = core // 4
        h0 = (core % 4) * HPC
        oc = res.results[core]["o"]  # [HPC, PD+1, SEQ]
        num = oc[:, :PD, :].astype(np.float64)
        den = oc[:, PD, :].astype(np.float64)
        for i in range(HPC):
            h = h0 + i
            bv = bkv[DIM + h * PD:DIM + (h + 1) * PD]
            O[b, h] = (num[i] / den[i][None, :]).T + bv[None, :]
    return O.reshape(B, SEQ, DIM)


# revision 14
# speedup vs baseline: 1.0582x; 1.0331x over previous
"""Cross-attention TRN2 Bass kernel (nn_CrossAttention).

Full-input contract: kernel(**inputs) takes the unsharded numpy inputs and
returns the full output. Internally shards across 8 NeuronCores:
  core c -> batch b = c // 4, heads h0 = (c % 4) * 4 .. h0+3  (B=2, H=16)

v2: the baseline was ACT-throughput-walled (~147us of exp on the scalar
engine).  This version splits softmax-exp across TWO engines per attention
unit's 16 d-tiles:
  - 11 tiles: exact exp on ACT (PSUM->SBUF bf16, ~1.03us/tile)
  - 5 tiles (DVE_D): Schraudolph exp in bf16 on DVE via one tensor_scalar:
      i16 = round(x * 2^7/ln2 + (127*2^7 - 5.59)); bitcast i16 -> bf16
    max rel err ~3.3% per element, ~1.3e-2 end-to-end (gate 2e-2).
  All ae tiles are bf16 so A@V uses the fast all-bf16 path (163ns/MM,
  vs 211 f32r).  Denominator rides as a 65th ones-column of v (free:
  A@V cost is N-bound).  12 tiles on ACT / DVE_D on DVE -- see DVE_D.
Inputs ship as fp16 (10 mantissa bits ~ f32r's 11; halves the DMA bytes --
the 8-core runs contend for shared HBM bandwidth, so DMA volume matters
more than the 1-core 72us suggests).  qT/kTp are also fp16, so every
matmul runs the 16-bit PE path (measured ~25% faster streams than f32r:
164 vs 211 ns per N=512 MM).  Weight tiles, x/y tiles and the yt pool are
double-buffered so iteration i+1's serial input-DMA queue is not blocked
behind iteration i's last weight use (the FIFO sync queue previously
serialized at ~78% of an iteration).
Output DMA triggers ride the otherwise-idle Pool queue -- on the ACT queue
they head-of-line-blocked the exp stream (each trigger waits on its DVE
copy, stalling every later exp; measured +80us).
Everything else follows the baseline: one serial input-DMA queue in priority
order, f32r q/k projections, row-paired K=64 QK^T, projection work
interleaved into the attention units to track DMA arrival.
Host: normalize by the denominator row, add v bias, reassemble the
reference's raw (B, H*Dy*pd) reshape.
"""

import numpy as np

import concourse.bacc as bacc
import concourse.tile as tile
from concourse import mybir
from concourse.bass_utils import run_bass_kernel_spmd

DIM = 1024
H = 16
B = 2
SEQ = 2048  # both SEQ_X and SEQ_Y
PD = 64  # head dim
HPC = 4  # heads per core
PCOLS = HPC * PD  # 256 projection columns per core
N_CORES = 8

F32 = mybir.dt.float32
F32R = mybir.dt.float32r
BF16 = mybir.dt.bfloat16
I16 = mybir.dt.int16
F16 = mybir.dt.float16

# d-tiles whose exp runs on DVE (Schraudolph); rest on ACT (exact)
DVE_D = (3, 6, 9, 12, 15)
OUT_Q = "gpsimd"  # engine queue for output DMA triggers
AEB_BUFS = 6
AEF_BUFS = 4
EXPA = float(2.0 ** 7 / np.log(2.0))
EXPB = float(127.0 * 2 ** 7 - 366393.0 / 65536.0)

_NC_CACHE = None


def _round_f32r(a: np.ndarray) -> np.ndarray:
    """Round fp32 -> float32r bit pattern (RNE, drop low 12 mantissa bits)."""
    b = np.ascontiguousarray(a, dtype=np.float32).view(np.uint32).astype(np.uint64)
    half = np.uint64(1 << 11)
    lsb_mask = np.uint64((1 << 12) - 1)
    rounded = (b + half - np.uint64(1) + ((b >> np.uint64(12)) & np.uint64(1))) & ~lsb_mask
    return rounded.astype(np.uint32).view(np.float32).reshape(a.shape)


def _build_nc(repeat=1, loop_n=0):
    nc = bacc.Bacc(trn_type="TRN2", name="cross_attention")

    yt = nc.dram_tensor("yt", [DIM, SEQ], F16, kind="ExternalInput")
    xt = nc.dram_tensor("xt", [DIM, SEQ], F16, kind="ExternalInput")
    wqt = nc.dram_tensor("wqt", [DIM, PCOLS], F16, kind="ExternalInput")
    wkt = nc.dram_tensor("wkt", [DIM, PCOLS], F16, kind="ExternalInput")
    wvt = nc.dram_tensor("wvt", [DIM, PCOLS], F16, kind="ExternalInput")
    bq = nc.dram_tensor("bq", [PCOLS, 1], F32, kind="ExternalInput")
    o = nc.dram_tensor("o", [HPC, PD + 1, SEQ], F32, kind="ExternalOutput")

    NC = DIM // 128  # 8 c-tiles
    ND = SEQ // 128  # 16 d-tiles
    YC = 512  # attention y-chunk
    NY = SEQ // YC  # 4 y-chunks
    HY = SEQ // 2  # 1024 (projection y-half)

    with tile.TileContext(nc) as tc:
        with (
            tc.tile_pool(name="persist", bufs=1) as pp,
            tc.tile_pool(name="wts", bufs=2) as wtp,
            tc.tile_pool(name="xtp", bufs=2) as xtp,
            tc.tile_pool(name="ytp", bufs=4) as ytp,
            tc.tile_pool(name="aeb", bufs=AEB_BUFS) as aebp,
            tc.tile_pool(name="aef", bufs=AEF_BUFS) as aefp,
            tc.tile_pool(name="outp", bufs=4) as outp,
            tc.tile_pool(name="ps_att", bufs=3, space="PSUM") as ps_att,
            tc.tile_pool(name="ps_o", bufs=2, space="PSUM") as ps_o,
        ):
            if loop_n:
                loop_cm = tc.For_i(0, loop_n, 1)
            else:
                loop_cm = None
            with (loop_cm if loop_cm is not None else __import__("contextlib").nullcontext()):
              for rep in range(repeat):
                # ---- resident loads ----
                # DMA behaves as ONE serial ~250GB/s resource: put every input
                # on the sync queue in exact priority order.  Outputs go on the
                # scalar queue.
                yt_src = yt.ap().rearrange("(c p) s -> p c s", p=128)
                xt_src = xt.ap().rearrange("(c p) s -> p c s", p=128)
                xt_e = [None] * 4

                def emit_xt(e):
                    t = xtp.tile([128, NC, 512], F16, tag=f"xte{e}", name=f"xte{e}_r{rep}")
                    nc.sync.dma_start(out=t, in_=xt_src[:, :, e * 512:(e + 1) * 512])
                    xt_e[e] = t

                wq_big = wtp.tile([128, NC, PCOLS], F16, tag="wq", name=f"wq_r{rep}")
                nc.sync.dma_start(
                    out=wq_big, in_=wqt.ap().rearrange("(c p) n -> p c n", p=128))
                # yt as four [128, 8, 512] y-chunk transfers; yt_cur2[yh][j]
                yt_cur2 = [[None, None], [None, None]]

                def emit_yt(yh, jj):
                    t = ytp.tile([128, NC, YC], F16, tag="yt", name=f"yt{yh}_{jj}_r{rep}")
                    lo = yh * HY + jj * YC
                    nc.sync.dma_start(out=t, in_=yt_src[:, :, lo:lo + YC])
                    yt_cur2[yh][jj] = t

                emit_yt(0, 0)
                wk_big = wtp.tile([128, NC, PCOLS], F16, tag="wk", name=f"wk_r{rep}")
                nc.sync.dma_start(
                    out=wk_big, in_=wkt.ap().rearrange("(c p) n -> p c n", p=128))
                emit_xt(0)
                emit_xt(1)
                bq_sb = []
                for m in range(2):
                    t = pp.tile([128, 1], F32, tag=f"bq{m}", name=f"bq{m}_r{rep}")
                    nc.sync.dma_start(out=t, in_=bq.ap()[m * 128:(m + 1) * 128, :])
                    bq_sb.append(t)
                wv_big = wtp.tile([128, NC, PCOLS], F16, tag="wv", name=f"wv_r{rep}")
                nc.sync.dma_start(
                    out=wv_big, in_=wvt.ap().rearrange("(c p) n -> p c n", p=128))
                emit_xt(2)
                emit_xt(3)
                emit_yt(0, 1)
                emit_yt(1, 0)
                emit_yt(1, 1)

                def xt_slice(c, lo, hi):
                    e = lo // 512
                    assert hi <= (e + 1) * 512
                    return xt_e[e][:, c, lo - e * 512:hi - e * 512]

                qT_sb = [pp.tile([128, SEQ], F16, tag=f"qT{m}", name=f"qT{m}_r{rep}") for m in range(2)]
                # kT per head pair: rows 0-63 = even head, 64-127 = odd head
                kTp = [pp.tile([128, SEQ], F16, tag=f"kp{m}", name=f"kp{m}_r{rep}") for m in range(2)]
                # v per d-tile, bf16, 65th column = 1.0 (denominator row)
                v_bf = [pp.tile([128, HPC, PD + 1], BF16, tag=f"vb{d}", name=f"vb{d}_r{rep}") for d in range(ND)]
                ones_sb = pp.tile([128, HPC], F32, tag="ones", name=f"ones_r{rep}")
                nc.vector.memset(ones_sb, 1.0)
                for d in range(ND):
                    nc.vector.tensor_copy(v_bf[d][:, :, PD:PD + 1], ones_sb)

                # Projection psum tiles share the "pa" slots of ps_att.
                def proj_q(m, yh, jj, cs=None, ps=None):
                    """q projection for one y-512 chunk jj of half yh."""
                    if ps is None:
                        ps = ps_att.tile([128, YC], F32, tag="pa", name=f"pq{m}_{yh}_{jj}_r{rep}")
                    for c in (cs if cs is not None else range(NC)):
                        nc.tensor.matmul(
                            ps,
                            wq_big[:, c, m * 128:(m + 1) * 128],
                            yt_cur2[yh][jj][:, c, :],
                            start=(c == 0),
                            stop=(c == NC - 1),
                        )
                    if cs is not None and list(cs)[-1] != NC - 1:
                        return ps
                    lo = yh * HY + jj * YC
                    nc.vector.tensor_scalar_add(qT_sb[m][:, lo:lo + YC], ps, bq_sb[m])
                    return ps

                def proj_split_q(m, yh, jj, d0, interleave, nq=2):
                    state = {"ps": None}
                    w = NC // nq

                    def chunk(i):
                        def f():
                            state["ps"] = proj_q(
                                m, yh, jj, cs=range(i * w, (i + 1) * w),
                                ps=state["ps"])
                        return f

                    for i in range(nq):
                        interleave.setdefault(d0 + i, []).append(chunk(i))

                def proj_pass_e(m, e, cs=None, ps=None):
                    """[128, 512] k projection chunk-pass: head pair m, x e-chunk."""
                    if ps is None:
                        ps = ps_att.tile([128, 512], F32, tag="pa", name=f"pk{m}_{e}_r{rep}")
                    for c in (cs if cs is not None else range(NC)):
                        nc.tensor.matmul(
                            ps,
                            wk_big[:, c, m * 128:(m + 1) * 128],
                            xt_e[e][:, c, :],
                            start=(c == 0),
                            stop=(c == NC - 1),
                        )
                    if cs is not None and list(cs)[-1] != NC - 1:
                        return ps
                    nc.vector.tensor_copy(kTp[m][:, e * 512:(e + 1) * 512], ps)
                    return ps

                def proj_pass(kind, m, yh):
                    proj_pass_e(m, 2 * yh)
                    proj_pass_e(m, 2 * yh + 1)

                def proj_split(kind, m, yh, d0, interleave, nq=4):
                    """Emit a k half-pass as nq chunk-passes at steps d0.."""
                    state = {"ps0": None, "ps1": None}

                    def chunk(e_off, i):
                        def f():
                            key = f"ps{e_off}"
                            state[key] = proj_pass_e(
                                m, 2 * yh + e_off, cs=range(i * 4, (i + 1) * 4),
                                ps=state[key])
                        return f

                    assert nq == 4
                    for i in range(2):
                        interleave.setdefault(d0 + i, []).append(chunk(0, i))
                    for i in range(2):
                        interleave.setdefault(d0 + 2 + i, []).append(chunk(1, i))

                def proj_v_single(d):
                    pvt = ps_att.tile([128, PCOLS], F32, tag="pa", name=f"pv{d}_r{rep}")
                    for c in range(NC):
                        nc.tensor.matmul(
                            pvt,
                            xt_slice(c, d * 128, (d + 1) * 128),
                            wv_big[:, c, :],
                            start=(c == 0),
                            stop=(c == NC - 1),
                        )
                    nc.vector.tensor_copy(
                        v_bf[d][:, :, 0:PD],
                        pvt.rearrange("p (h e) -> p h e", h=HPC),
                    )

                def attention_unit(m, yc, interleave=None, av_lag=4):
                    """One (head pair, y-chunk of 512): row-tiled QK^T -> exp
                    (ACT or DVE by d-tile) -> A@V for both heads."""
                    po_a = ps_o.tile([PD + 1, YC], F32, tag="po", name=f"poA{m}_{yc}_r{rep}")
                    po_b = ps_o.tile([PD + 1, YC], F32, tag="po", name=f"poB{m}_{yc}_r{rep}")
                    y0 = yc * YC
                    ae_q = {}

                    def emit_av(d):
                        v = v_bf[d]
                        nc.tensor.matmul(
                            po_a,
                            v[:, 2 * m, :],
                            ae_q[d][:, 0:YC],
                            start=(d == 0), stop=(d == ND - 1),
                        )
                        nc.tensor.matmul(
                            po_b,
                            v[:, 2 * m + 1, :],
                            ae_q[d][:, YC:2 * YC],
                            start=(d == 0), stop=(d == ND - 1),
                        )
                        del ae_q[d]

                    for d in range(ND):
                        if interleave and d in interleave:
                            fns = interleave[d]
                            for fn in (fns if isinstance(fns, (list, tuple)) else [fns]):
                                fn()
                        pa = ps_att.tile([128, 2 * YC], F32, tag="pa", name=f"pa{m}_{d}_{yc}_r{rep}")
                        nc.tensor.matmul(
                            pa[:, 0:YC],
                            kTp[m][0:64, d * 128:(d + 1) * 128],
                            qT_sb[m][0:64, y0:y0 + YC],
                            start=True, stop=True,
                        )
                        nc.tensor.matmul(
                            pa[:, YC:2 * YC],
                            kTp[m][64:128, d * 128:(d + 1) * 128],
                            qT_sb[m][64:128, y0:y0 + YC],
                            start=True, stop=True,
                        )
                        if d in DVE_D:
                            ae = aefp.tile([128, 2 * YC], BF16, tag="aef", name=f"aef{m}_{d}_{yc}_r{rep}")
                            nc.vector.tensor_scalar(
                                ae.bitcast(I16), pa, EXPA, EXPB,
                                mybir.AluOpType.mult, mybir.AluOpType.add)
                        else:
                            ae = aebp.tile([128, 2 * YC], BF16, tag="aeb", name=f"aeb{m}_{d}_{yc}_r{rep}")
                            nc.scalar.activation(
                                out=ae,
                                in_=pa,
                                func=mybir.ActivationFunctionType.Exp,
                                scale=1.0,
                            )
                        ae_q[d] = ae
                        if d >= av_lag:
                            emit_av(d - av_lag)
                    for d in range(ND - av_lag, ND):
                        emit_av(d)
                    for j, po in ((0, po_a), (1, po_b)):
                        h = 2 * m + j
                        osb = outp.tile([PD + 1, YC], F32, tag="osb", name=f"osb{h}_{yc}_r{rep}")
                        nc.vector.tensor_copy(osb, po)
                        getattr(nc, OUT_Q).dma_start(
                            out=o.ap()[h, :, y0:y0 + YC], in_=osb)

                # ---- emission order drives scheduling priority ----
                # Unit order (0,0),(1,0),(0,1),(1,1),... so work needing the
                # second x half / second y chunks lands after its DMA.
                proj_q(0, 0, 0)
                proj_q(1, 0, 0)
                proj_pass("k", 0, 0)
                il0 = {}
                for d in range(8):
                    il0.setdefault(d, []).append(lambda d=d: proj_v_single(d))
                proj_split("k", 0, 1, 4, il0)      # x half 1 (xt1)
                proj_split("k", 1, 0, 8, il0)      # x half 0
                for d in range(8, 13):
                    il0.setdefault(d + 3, []).append(lambda d=d: proj_v_single(d))
                for d in range(13, ND):
                    il0.setdefault(15, []).append(lambda d=d: proj_v_single(d))
                attention_unit(0, 0, interleave=il0, av_lag=3)
                il1 = {}
                proj_split("k", 1, 1, 0, il1)      # x half 1
                proj_split_q(0, 0, 1, 4, il1)      # y chunk 1 (yt0 j1)
                proj_split_q(1, 0, 1, 6, il1)
                attention_unit(1, 0, interleave=il1)
                il2 = {}
                proj_split_q(0, 1, 0, 2, il2)      # y half 1 (yt1 j0)
                proj_split_q(1, 1, 0, 4, il2)
                attention_unit(0, 1, interleave=il2)
                il3 = {}
                proj_split_q(0, 1, 1, 2, il3)
                proj_split_q(1, 1, 1, 4, il3)
                attention_unit(1, 1, interleave=il3)
                attention_unit(0, 2)
                attention_unit(1, 2)
                attention_unit(0, 3)
                attention_unit(1, 3)

    nc.compile()
    return nc


def _get_nc():
    global _NC_CACHE
    if _NC_CACHE is None:
        _NC_CACHE = _build_nc()
    return _NC_CACHE


_NC_LOOP_CACHE = {}


def _get_nc_loop(loop_n):
    if loop_n not in _NC_LOOP_CACHE:
        _NC_LOOP_CACHE[loop_n] = _build_nc(1, loop_n=loop_n)
    return _NC_LOOP_CACHE[loop_n]


def make_in_map(x, y, Wq, bq_np, Wkv, core):
    b = core // 4
    h0 = (core % 4) * HPC
    cs = slice(h0 * PD, h0 * PD + PCOLS)
    vs = slice(DIM + h0 * PD, DIM + h0 * PD + PCOLS)
    return {
        "yt": np.ascontiguousarray(y[b].T).astype(np.float16),
        "xt": np.ascontiguousarray(x[b].T).astype(np.float16),
        "wqt": np.ascontiguousarray(Wq[cs, :].T).astype(np.float16),
        "wkt": np.ascontiguousarray(Wkv[cs, :].T).astype(np.float16),
        "wvt": np.ascontiguousarray(Wkv[vs, :].T).astype(np.float16),
        "bq": np.ascontiguousarray(bq_np[cs].reshape(PCOLS, 1)).astype(np.float32),
    }


def kernel(x, y, Wq, bq, Wkv, bkv, _collect_results=None):
    x = np.asarray(x, dtype=np.float32)
    y = np.asarray(y, dtype=np.float32)
    Wq = np.asarray(Wq, dtype=np.float32)
    bq = np.asarray(bq, dtype=np.float32)
    Wkv = np.asarray(Wkv, dtype=np.float32)
    bkv = np.asarray(bkv, dtype=np.float32)

    nc = _get_nc()

    in_maps = [make_in_map(x, y, Wq, bq, Wkv, core) for core in range(N_CORES)]

    res = run_bass_kernel_spmd(nc, in_maps, list(range(N_CORES)))
    if _collect_results is not None:
        _collect_results.append(res)

    O = np.empty((B, H, SEQ, PD), np.float32)
    for core in range(N_CORES):
        b = core // 4
        h0 = (core % 4) * HPC
        oc = res.results[core]["o"]  # [HPC, PD+1, SEQ]
        num = oc[:, :PD, :].astype(np.float64)
        den = oc[:, PD, :].astype(np.float64)
        for i in range(HPC):
            h = h0 + i
            bv = bkv[DIM + h * PD:DIM + (h + 1) * PD]
            O[b, h] = (num[i] / den[i][None, :]).T + bv[None, :]
    return O.reshape(B, SEQ, DIM)
